# revision 11
# baseline (speedup 1.0000x reference)
"""Multi-head attention (B=2, S=2048, D=1024, H=16) on 8 Trainium2 cores.

Sharding: data-parallel over the 2 batches x tensor-parallel over 4 groups
of 4 heads.  Core c handles batch c//4 and heads [4*(c%4) : 4*(c%4)+4]
(columns [256*(c%4) : +256] of Wk/Wv, same rows of Wo).  Each core produces
a partial [S, D] output (its heads' contribution to o @ Wo); the host sums
the 4 partials per batch.

Per-core dataflow (everything fp32 storage, float32r matmuls):
  qT,kT,vT [D,S] (host-pre-transposed) --proj--> QT,KT [128,2,S] (head-major
  rows), V [sk,hd] with an extra ones column.  Attention per head in
  "scores-transposed" layout [sk_part, sq_free]: scoresT = KT_j^T @ QT,
  causal diagonal handled by a bf16 identity-matmul adding a -480
  lower-triangular tile into PSUM, exp on ScalarE (scale folded in,
  no max subtraction - scores are O(6) for sane inputs), then
  UT[65, S] += Vaug_j^T @ expT accumulated in PSUM where row 64 = softmax
  denominators (from the ones column).  Normalize: recip + PE broadcast +
  DVE multiply into oT [d_part, sq].  Final: out = oT^T @ Wo per 128-row
  block, DMA to HBM.
"""

import itertools
from contextlib import ExitStack

import numpy as np

import concourse.bass as bass
import concourse.tile as tile
from concourse import bacc, bass_utils, mybir
from concourse.masks import make_identity

B, S, D, H = 2, 2048, 1024, 16
HD = D // H            # 64
NCORES = 8
HPC = 4                # heads per core
CW = HPC * HD          # 256 weight cols per core
NCH = 4                # sequence chunks of 512
MASKVAL = -480.0       # additive pre-scale causal mask value (exp -> ~e-60)
S_INV = float(1.0 / (np.sqrt(np.float32(HD)) + np.float32(1e-8)))

F32 = mybir.dt.float32
F32R = mybir.dt.float32r
BF16 = mybir.dt.bfloat16


def _build(mode: str, bias_k: bool, bias_v: bool, bias_o: bool = False):
    del bias_o  # bo is added host-side in assemble()
    """Build + compile the SPMD program.  mode: 'causal' | 'none' | 'general'."""
    nc = bacc.Bacc("TRN2", target_bir_lowering=False, debug=False,
                   num_devices=NCORES)

    qT_d = nc.dram_tensor("qT", [D, S], F32R, kind="ExternalInput").ap()
    kT_d = nc.dram_tensor("kT", [D, S], F32R, kind="ExternalInput").ap()
    vT_d = nc.dram_tensor("vT", [D, S], F32R, kind="ExternalInput").ap()
    wk_d = nc.dram_tensor("wk", [D, CW], F32R, kind="ExternalInput").ap()
    wv_d = nc.dram_tensor("wv", [D, CW], F32R, kind="ExternalInput").ap()
    wo_d = nc.dram_tensor("wo", [CW, D], F32R, kind="ExternalInput").ap()
    bk_d = nc.dram_tensor("bk", [1, CW], F32R, kind="ExternalInput").ap() if bias_k else None
    bv_d = nc.dram_tensor("bv", [1, CW], F32R, kind="ExternalInput").ap() if bias_v else None
    maskT_d = (nc.dram_tensor("maskT", [S, S], BF16, kind="ExternalInput").ap()
               if mode == "general" else None)
    vones_d = nc.dram_tensor("vones", [128, 16], F32R, kind="ExternalInput").ap()
    ones1_d = (nc.dram_tensor("ones1", [1, 512], F32R, kind="ExternalInput").ap()
               if (bias_k or bias_v) else None)
    out_d = nc.dram_tensor("out", [S, D], F32, kind="ExternalOutput").ap()

    with tile.TileContext(nc) as tc, ExitStack() as ctx:
        sb1 = ctx.enter_context(tc.tile_pool(name="persist", bufs=1))
        qt_pool = ctx.enter_context(tc.tile_pool(name="qt", bufs=NCH))
        kt_pool = ctx.enter_context(tc.tile_pool(name="kt", bufs=NCH))
        v_pool = ctx.enter_context(tc.tile_pool(name="v", bufs=NCH))
        stage_pool = ctx.enter_context(tc.tile_pool(name="stage", bufs=3))
        exp_pool = ctx.enter_context(tc.tile_pool(name="exp", bufs=3))
        recip_pool = ctx.enter_context(tc.tile_pool(name="recip", bufs=1))
        ottmp_pool = ctx.enter_context(tc.tile_pool(name="ottmp", bufs=1))
        bcast_pool = ctx.enter_context(tc.tile_pool(name="bcast", bufs=1))
        outsb_pool = ctx.enter_context(tc.tile_pool(name="outsb", bufs=2))
        ps_pool = ctx.enter_context(tc.tile_pool(name="ps", bufs=2, space="PSUM"))
        ut_pool = ctx.enter_context(tc.tile_pool(name="ut", bufs=1, space="PSUM"))
        if mode == "general":
            mask_pool = ctx.enter_context(tc.tile_pool(name="mask", bufs=3))

        # ---- constants / weights -------------------------------------
        wk_sb = sb1.tile([128, 8, CW], F32R)
        nc.sync.dma_start(wk_sb[:], wk_d.rearrange("(c p) n -> p c n", p=128))
        wv_sb = sb1.tile([128, 8, CW], F32R)
        nc.sync.dma_start(wv_sb[:], wv_d.rearrange("(c p) n -> p c n", p=128))
        wo_sb = sb1.tile([128, 2, D], F32R)
        nc.sync.dma_start(wo_sb[:], wo_d.rearrange("(m p) n -> p m n", p=128))
        if bias_k or bias_v:
            ones_sb = sb1.tile([1, 512], F32R)
            nc.sync.dma_start(ones_sb[:], ones1_d[:])
        if bias_k:
            bk_sb = sb1.tile([1, CW], F32R)
            nc.sync.dma_start(bk_sb[:], bk_d[:])
        if bias_v:
            bv_sb = sb1.tile([1, CW], F32R)
            nc.sync.dma_start(bv_sb[:], bv_d[:])
        if mode != "none":
            ident = sb1.tile([128, 128], BF16)
            make_identity(nc, ident[:])
        if mode == "causal":
            # dmask[p, f] = MASKVAL where f < p (sq < sk), else 0
            dmask = sb1.tile([128, 128], BF16)
            nc.gpsimd.memset(dmask[:], 0.0)
            nc.gpsimd.affine_select(
                out=dmask[:], in_=dmask[:],
                compare_op=mybir.AluOpType.is_ge,
                fill=MASKVAL, base=0,
                pattern=[[1, 128]], channel_multiplier=-1,
            )

        # V tiles: [128 sk, 4 blk, 4 head, 66] - col 64 is the ones column
        v_tiles = [v_pool.tile([128, 4, HPC, 66], F32R, tag="v", name=f"v{c}") for c in range(NCH)]
        for c in range(NCH):
            nc.sync.dma_start(v_tiles[c][:, :, :, 64:65],
                              vones_d[:].rearrange("p (b h e) -> p b h e", b=4, h=HPC))
        qt_tiles = [qt_pool.tile([128, 2, 512], F32R, tag="qt", name=f"qt{c}") for c in range(NCH)]
        kt_tiles = [kt_pool.tile([128, 2, 512], F32R, tag="kt", name=f"kt{c}") for c in range(NCH)]
        oT_sb = sb1.tile([128, 2, S], F32R)

        copy_engines = itertools.cycle([nc.scalar, nc.vector])

        def ps_copy(dst, src):
            eng = next(copy_engines)
            if eng is nc.scalar:
                nc.scalar.copy(dst, src)
            else:
                nc.vector.tensor_copy(dst, src)

        # ---- phase 1: projections ------------------------------------
        for c in range(NCH):
            sl = bass.ds(c * 512, 512)
            kst = stage_pool.tile([128, 8, 512], F32R, tag="stage")
            nc.sync.dma_start(kst[:], kT_d.rearrange("(cc p) s -> p cc s", p=128)[:, :, sl])
            vst = stage_pool.tile([128, 8, 512], F32R, tag="stage")
            nc.sync.dma_start(vst[:], vT_d.rearrange("(cc p) s -> p cc s", p=128)[:, :, sl])
            qst = stage_pool.tile([128, 8, 512], F32R, tag="stage")
            nc.sync.dma_start(qst[:], qT_d.rearrange("(cc p) s -> p cc s", p=128)[:, :, sl])

            # KT / QT projections (transposed layout, 2 m-halves of 128)
            for name, st, wsb, bsb, dst in (
                ("k", kst, wk_sb, bk_sb if bias_k else None, kt_tiles[c]),
                ("q", qst, wk_sb, bk_sb if bias_k else None, qt_tiles[c]),
            ):
                ps = ps_pool.tile([128, 1024], F32, tag="ps")
                for m in range(2):
                    reg = ps[:, bass.ds(m * 512, 512)]
                    first = True
                    if bsb is not None:
                        nc.tensor.matmul(reg, (bsb[0:1, bass.ds(m * 128, 128)]),
                                         (ones_sb[0:1, :]), start=True, stop=False)
                        first = False
                    for dc in range(8):
                        nc.tensor.matmul(
                            reg,
                            (wsb[:, dc, bass.ds(m * 128, 128)]),
                            (st[:, dc, :]),
                            start=first, stop=(dc == 7))
                        first = False
                ps_copy(dst[:, :, :], ps[:].rearrange("p (m s) -> p m s", m=2))

            # V projection (natural layout)
            psv = ps_pool.tile([128, 1024], F32, tag="ps")
            for blk in range(4):
                reg = psv[:, bass.ds(blk * 256, 256)]
                first = True
                if bias_v:
                    nc.tensor.matmul(reg, (ones_sb[0:1, 0:128]), (bv_sb[0:1, :]),
                                     start=True, stop=False)
                    first = False
                for dc in range(8):
                    nc.tensor.matmul(
                        reg,
                        (vst[:, dc, bass.ds(blk * 128, 128)]),
                        (wv_sb[:, dc, :]),
                        start=first, stop=(dc == 7))
                    first = False
            ps_copy(v_tiles[c][:, :, :, 0:64],
                    psv[:].rearrange("p (b h e) -> p b h e", b=4, h=HPC))

        # ---- phase 2: attention per head -----------------------------
        full_grid = mode != "causal"

        for hl in range(HPC):
            m = hl // 2
            p0 = 64 * (hl % 2)
            ut = ut_pool.tile([128, S], F32, tag="ut")

            if full_grid:
                steps = [(j, w) for j in range(16) for w in range(2)]
            else:
                steps = [(j, w) for j in range(16) for w in range(j // 8, 2)]

            win_ps = {}
            win_exp = {}

            def emit_scores(t):
                j, w = t
                ps = ps_pool.tile([128, 1024], F32, tag="ps")
                win_ps[t] = ps
                a0 = 1024 * w if full_grid else max(128 * j, 1024 * w)
                if mode == "general":
                    mt = mask_pool.tile([128, 1024], BF16, tag="mask")
                    nc.sync.dma_start(
                        mt[:, a0 - 1024 * w:],
                        maskT_d[bass.ds(128 * j, 128), bass.ds(a0, 1024 * (w + 1) - a0)])
                for sl_i in range(2):
                    lo = 1024 * w + 512 * sl_i
                    hi = lo + 512
                    if hi <= a0:
                        continue
                    nlo = max(lo, a0)
                    reg = ps[:, bass.ds(nlo - 1024 * w, hi - nlo)]
                    cq, off = nlo // 512, nlo % 512
                    rhs = qt_tiles[cq][p0:p0 + 64, m, bass.ds(off, hi - nlo)]
                    lhsT = kt_tiles[j // 4][p0:p0 + 64, m, bass.ds(128 * (j % 4), 128)]
                    diag_here = (mode == "causal") and lo <= 128 * j < hi
                    mask_here = (mode == "general")
                    nc.tensor.matmul(reg, (lhsT), (rhs), start=True,
                                     stop=not (diag_here or mask_here))
                    if diag_here:
                        doff = 128 * j - 1024 * w
                        nc.tensor.matmul(ps[:, bass.ds(doff, 128)], ident[:], dmask[:],
                                         start=False, stop=True)
                    elif mask_here:
                        nc.tensor.matmul(reg, ident[:],
                                         mt[:, bass.ds(nlo - 1024 * w, hi - nlo)],
                                         start=False, stop=True)

            def emit_exp(t):
                j, w = t
                ps = win_ps[t]
                a0 = 1024 * w if full_grid else max(128 * j, 1024 * w)
                off = a0 - 1024 * w
                et = exp_pool.tile([128, 1024], F32R, tag="exp")
                win_exp[t] = et
                nc.scalar.activation(et[:, off:1024], ps[:, off:1024],
                                     mybir.ActivationFunctionType.Exp, scale=S_INV)

            def emit_pv(t):
                j, w = t
                et = win_exp.pop(t)
                win_ps.pop(t)
                a0 = 1024 * w if full_grid else max(128 * j, 1024 * w)
                for sl_i in range(2):
                    lo = 1024 * w + 512 * sl_i
                    hi = lo + 512
                    if hi <= a0:
                        continue
                    nlo = max(lo, a0)
                    r = nlo // 512  # absolute 512-col region of UT
                    if full_grid:
                        start, stop = (j == 0), (j == 15)
                    else:
                        start, stop = (j == 0), (j == 4 * r + 3)
                    nc.tensor.matmul(
                        ut[0:65, bass.ds(nlo, hi - nlo)],
                        (v_tiles[j // 4][:, j % 4, hl, 0:65]),
                        (et[:, bass.ds(nlo - 1024 * w, hi - nlo)]),
                        start=start, stop=stop)

            emit_scores(steps[0])
            for i, t in enumerate(steps):
                if i + 1 < len(steps):
                    emit_scores(steps[i + 1])
                emit_exp(t)
                emit_pv(t)

            # normalize: oT[hd, sq] = UT[0:64] * (1 / UT[64])
            rc = recip_pool.tile([1, S], F32, tag="recip")
            nc.vector.reciprocal(rc[:], ut[64:65, :])
            bc = bcast_pool.tile([64, S], F32, tag="bcast")
            nc.gpsimd.partition_broadcast(bc[:], rc[:], channels=64)
            if p0 == 0:
                dst = oT_sb[0:64, m, :]
            else:
                ott = ottmp_pool.tile([64, S], F32R, tag="ottmp")
                dst = ott[:, :]
            for half in range(2):
                nc.vector.tensor_mul(
                    dst[:, bass.ds(1024 * half, 1024)],
                    ut[0:64, bass.ds(1024 * half, 1024)],
                    bc[:, bass.ds(1024 * half, 1024)])
            if p0:
                nc.sync.dma_start(oT_sb[64:128, m, :], ott[:, :])

        # ---- phase 3: output projection ------------------------------
        for sb in range(16):
            ps = ps_pool.tile([128, 1024], F32, tag="ps")
            for nh in range(2):
                reg = ps[:, bass.ds(nh * 512, 512)]
                first = True
                for m in range(2):
                    nc.tensor.matmul(
                        reg,
                        (oT_sb[:, m, bass.ds(sb * 128, 128)]),
                        (wo_sb[:, m, bass.ds(nh * 512, 512)]),
                        start=first, stop=(m == 1))
                    first = False
            ob = outsb_pool.tile([128, D], F32, tag="outsb")
            ps_copy(ob[:], ps[:])
            nc.sync.dma_start(out_d[bass.ds(sb * 128, 128), :], ob[:])

    nc.compile()
    return nc


_VONES = np.ones((128, 16), dtype=np.float32)
_ONES1 = np.ones((1, 512), dtype=np.float32)

_CACHE = {}


def _get_nc(mode, bias_k, bias_v, bias_o):
    key = (mode, bias_k, bias_v, bias_o)
    if key not in _CACHE:
        _CACHE[key] = _build(mode, bias_k, bias_v, bias_o)
    return _CACHE[key]


def make_in_maps(q, k, v, mask, Wk, bk, Wv, bv, Wo, bo):
    """Host-side sharding. Returns (mode, bias flags, in_maps)."""
    import ml_dtypes

    q = np.asarray(q, dtype=np.float32)
    k = np.asarray(k, dtype=np.float32)
    v = np.asarray(v, dtype=np.float32)
    Wk = np.asarray(Wk, dtype=np.float32)
    Wv = np.asarray(Wv, dtype=np.float32)
    Wo = np.asarray(Wo, dtype=np.float32)
    bk = np.asarray(bk, dtype=np.float32).reshape(-1)
    bv = np.asarray(bv, dtype=np.float32).reshape(-1)
    bo = np.asarray(bo, dtype=np.float32).reshape(-1)
    mask2d = np.asarray(mask, dtype=np.float32).reshape(S, S)

    if not mask2d.any():
        mode = "none"
    elif np.array_equal(mask2d, np.triu(np.ones((S, S), np.float32), 1)):
        mode = "causal"
    else:
        mode = "general"
    bias_k, bias_v, bias_o = bool(bk.any()), bool(bv.any()), bool(bo.any())

    qT = [np.ascontiguousarray(q[b].T) for b in range(B)]
    kT = [np.ascontiguousarray(k[b].T) for b in range(B)]
    vT = [np.ascontiguousarray(v[b].T) for b in range(B)]
    if mode == "general":
        # pre-scale so adding before the fused exp scale matches the
        # reference's post-scale add:  (raw + m)*S_INV == raw*S_INV + mask*(-1e9)
        maskT = np.ascontiguousarray(
            (mask2d.T * np.float32(-1e9 / S_INV)).astype(ml_dtypes.bfloat16))

    in_maps = []
    for core in range(NCORES):
        b, g = divmod(core, HPC)
        cs = slice(CW * g, CW * (g + 1))
        im = {
            "qT": qT[b], "kT": kT[b], "vT": vT[b],
            "wk": np.ascontiguousarray(Wk[:, cs]),
            "wv": np.ascontiguousarray(Wv[:, cs]),
            "wo": np.ascontiguousarray(Wo[cs, :]),
        }
        im["vones"] = _VONES
        if bias_k or bias_v:
            im["ones1"] = _ONES1
        if bias_k:
            im["bk"] = np.ascontiguousarray(bk[cs]).reshape(1, CW)
        if bias_v:
            im["bv"] = np.ascontiguousarray(bv[cs]).reshape(1, CW)
        if mode == "general":
            im["maskT"] = maskT
        in_maps.append(im)
    return mode, (bias_k, bias_v, bias_o), in_maps


def assemble(results, bo=None):
    """Sum per-core partial outputs into the full [B, S, D] output."""
    full = np.zeros((B, S, D), dtype=np.float32)
    for b in range(B):
        acc = results[4 * b]["out"].astype(np.float32)
        for c in range(4 * b + 1, 4 * b + 4):
            acc = acc + results[c]["out"]
        if bo is not None:
            acc = acc + bo
        full[b] = acc
    return full


def kernel(q, k, v, mask, Wk, bk, Wv, bv, Wo, bo):
    mode, (bias_k, bias_v, bias_o), in_maps = make_in_maps(
        q, k, v, mask, Wk, bk, Wv, bv, Wo, bo)
    nc = _get_nc(mode, bias_k, bias_v, bias_o)
    res = bass_utils.run_bass_kernel_spmd(nc, in_maps, core_ids=list(range(NCORES)))
    bo_arr = np.asarray(bo, dtype=np.float32).reshape(-1) if bias_o else None
    return assemble(res.results, bo_arr)


# revision 12
# speedup vs baseline: 1.2823x; 1.2823x over previous
"""Multi-head attention (B=2, S=2048, D=1024, H=16) on 8 Trainium2 cores.

Sharding: data-parallel over the 2 batches x tensor-parallel over 4 groups
of 4 heads.  Core c handles batch c//4 and heads [4*(c%4) : 4*(c%4)+4]
(columns [256*(c%4) : +256] of Wk/Wv, same rows of Wo).  Each core produces
a partial [S, D] output (its heads' contribution to o @ Wo); the host sums
the 4 partials per batch (and adds bo once).

Per-core dataflow (bf16 matmul operands, fp32 PSUM accumulation):
  qT,kT,vT [D,S] fp32 (host-pre-transposed) are DMA-cast to bf16 on load.
  Projections produce QT,KT [128,2,S] (head-major rows) and V [sk,hd] with
  an extra ones column.  Attention per head in "scores-transposed" layout
  [sk_part, sq_free]: scoresT = KT_j^T @ QT; the causal diagonal adds a
  bf16 -480 lower-triangular tile into PSUM via an identity matmul; exp on
  ScalarE (scale folded in; no max subtraction - scores are O(6));
  UT[65, S] += Vaug_j^T @ expT accumulated in PSUM, row 64 = softmax
  denominators (from the ones column).  Normalization is region-wise
  (512 cols at a time, as soon as that region's last k-block lands):
  sums -> DMA reshape [1,512]->[128,4] -> cheap DVE reciprocal -> DMA back
  -> gpsimd partition_broadcast -> one DVE multiply into oT [d_part, sq].
  Final: out = oT^T @ Wo per 128-row block, fp32 DMA to HBM.
"""

import itertools
import os
from contextlib import ExitStack

import numpy as np

import concourse.bass as bass
import concourse.tile as tile
from concourse import bacc, bass_utils, mybir
from concourse.masks import make_identity

B, S, D, H = 2, 2048, 1024, 16
HD = D // H            # 64
NCORES = 8
HPC = 4                # heads per core
CW = HPC * HD          # 256 weight cols per core
NCH = 4                # sequence chunks of 512
MASKVAL = -480.0       # additive pre-scale causal mask value (exp -> ~e-60)
S_INV = float(1.0 / (np.sqrt(np.float32(HD)) + np.float32(1e-8)))

F32 = mybir.dt.float32
F32R = mybir.dt.float32r
BF16 = mybir.dt.bfloat16


def _build(mode: str, bias_k: bool, bias_v: bool, precision: str = "bf16"):
    """Build + compile the SPMD program.

    mode: 'causal' | 'none' | 'general'
    precision: 'bf16' (everything bf16) or 'mixed' (fp32r projections).
    """
    nc = bacc.Bacc("TRN2", target_bir_lowering=False, debug=False,
                   num_devices=NCORES)
    xdt = BF16 if precision == "bf16" else F32R
    in_dt = F32 if precision == "bf16" else F32R  # dram decl for x/w inputs

    qT_d = nc.dram_tensor("qT", [D, S], in_dt, kind="ExternalInput").ap()
    kT_d = nc.dram_tensor("kT", [D, S], in_dt, kind="ExternalInput").ap()
    vT_d = nc.dram_tensor("vT", [D, S], in_dt, kind="ExternalInput").ap()
    wk_d = nc.dram_tensor("wk", [D, CW], in_dt, kind="ExternalInput").ap()
    wv_d = nc.dram_tensor("wv", [D, CW], in_dt, kind="ExternalInput").ap()
    wo_d = nc.dram_tensor("wo", [CW, D], F32, kind="ExternalInput").ap()
    bk_d = nc.dram_tensor("bk", [1, CW], in_dt, kind="ExternalInput").ap() if bias_k else None
    bv_d = nc.dram_tensor("bv", [1, CW], in_dt, kind="ExternalInput").ap() if bias_v else None
    maskT_d = (nc.dram_tensor("maskT", [S, S], BF16, kind="ExternalInput").ap()
               if mode == "general" else None)
    vones_d = nc.dram_tensor("vones", [128, 16], BF16, kind="ExternalInput").ap()
    ones1_d = (nc.dram_tensor("ones1", [1, 512], xdt, kind="ExternalInput").ap()
               if (bias_k or bias_v) else None)
    out_d = nc.dram_tensor("out", [S, D], F32, kind="ExternalOutput").ap()

    def load(dst, src):
        """DMA load, casting via SWDGE when dtypes differ."""
        if dst.dtype != src.dtype:
            nc.gpsimd.dma_start(dst, src)
        else:
            nc.sync.dma_start(dst, src)

    with tile.TileContext(nc) as tc, ExitStack() as ctx:
        sb1 = ctx.enter_context(tc.tile_pool(name="persist", bufs=1))
        qt_pool = ctx.enter_context(tc.tile_pool(name="qt", bufs=NCH))
        kt_pool = ctx.enter_context(tc.tile_pool(name="kt", bufs=NCH))
        v_pool = ctx.enter_context(tc.tile_pool(name="v", bufs=NCH))
        stage_pool = ctx.enter_context(tc.tile_pool(name="stage", bufs=4))
        exp_pool = ctx.enter_context(tc.tile_pool(name="exp", bufs=3))
        sums_pool = ctx.enter_context(tc.tile_pool(name="sums", bufs=4))
        srt_pool = ctx.enter_context(tc.tile_pool(name="srt", bufs=4))
        rcb_pool = ctx.enter_context(tc.tile_pool(name="rcb", bufs=4))
        bc_pool = ctx.enter_context(tc.tile_pool(name="bc", bufs=5))
        ottmp_pool = ctx.enter_context(tc.tile_pool(name="ottmp", bufs=1))
        outsb_pool = ctx.enter_context(tc.tile_pool(name="outsb", bufs=2))
        ps_pool = ctx.enter_context(tc.tile_pool(name="ps", bufs=2, space="PSUM"))
        ut_pool = ctx.enter_context(tc.tile_pool(name="ut", bufs=1, space="PSUM"))
        if mode == "general":
            mask_pool = ctx.enter_context(tc.tile_pool(name="mask", bufs=3))

        # ---- constants / weights -------------------------------------
        wk_sb = sb1.tile([128, 8, CW], xdt)
        load(wk_sb[:], wk_d.rearrange("(c p) n -> p c n", p=128))
        wv_sb = sb1.tile([128, 8, CW], xdt)
        load(wv_sb[:], wv_d.rearrange("(c p) n -> p c n", p=128))
        wo_sb = sb1.tile([128, 2, D], BF16)
        load(wo_sb[:], wo_d.rearrange("(m p) n -> p m n", p=128))
        if bias_k:
            bk_sb = sb1.tile([1, CW], xdt)
            load(bk_sb[:], bk_d[:])
        if bias_v:
            bv_sb = sb1.tile([1, CW], xdt)
            load(bv_sb[:], bv_d[:])
        if bias_k or bias_v:
            ones_sb = sb1.tile([1, 512], xdt)
            nc.sync.dma_start(ones_sb[:], ones1_d[:])
        if mode != "none":
            ident = sb1.tile([128, 128], BF16)
            make_identity(nc, ident[:])
        if mode == "causal":
            # dmask[p, f] = MASKVAL where f < p (sq < sk), else 0
            dmask = sb1.tile([128, 128], BF16)
            nc.gpsimd.memset(dmask[:], 0.0)
            nc.gpsimd.affine_select(
                out=dmask[:], in_=dmask[:],
                compare_op=mybir.AluOpType.is_ge,
                fill=MASKVAL, base=0,
                pattern=[[1, 128]], channel_multiplier=-1,
            )

        # V tiles: [128 sk, 4 blk, 4 head, 66] - col 64 is the ones column
        v_tiles = [v_pool.tile([128, 4, HPC, 66], BF16, tag="v", name=f"v{c}")
                   for c in range(NCH)]
        for c in range(NCH):
            nc.sync.dma_start(v_tiles[c][:, :, :, 64:65],
                              vones_d[:].rearrange("p (b h e) -> p b h e", b=4, h=HPC))
        qt_tiles = [qt_pool.tile([128, 2, 512], BF16, tag="qt", name=f"qt{c}")
                    for c in range(NCH)]
        kt_tiles = [kt_pool.tile([128, 2, 512], BF16, tag="kt", name=f"kt{c}")
                    for c in range(NCH)]
        oT_sb = sb1.tile([128, 2, S], BF16)

        copy_engines = itertools.cycle([nc.scalar, nc.vector])

        def ps_copy(dst, src):
            eng = next(copy_engines)
            if eng is nc.scalar:
                nc.scalar.copy(dst, src)
            else:
                nc.vector.tensor_copy(dst, src)

        # ---- phase 1: projections ------------------------------------
        for c in range(NCH):
            sl = bass.ds(c * 512, 512)
            kst = stage_pool.tile([128, 8, 512], xdt, tag="stage", name=f"kst{c}")
            load(kst[:], kT_d.rearrange("(cc p) s -> p cc s", p=128)[:, :, sl])
            vst = stage_pool.tile([128, 8, 512], xdt, tag="stage", name=f"vst{c}")
            load(vst[:], vT_d.rearrange("(cc p) s -> p cc s", p=128)[:, :, sl])
            qst = stage_pool.tile([128, 8, 512], xdt, tag="stage", name=f"qst{c}")
            load(qst[:], qT_d.rearrange("(cc p) s -> p cc s", p=128)[:, :, sl])

            # KT / QT projections (transposed layout, 2 m-halves of 128)
            for st, dst in ((kst, kt_tiles[c]), (qst, qt_tiles[c])):
                ps = ps_pool.tile([128, 1024], F32, tag="ps", name=f"psp{c}")
                for m in range(2):
                    reg = ps[:, bass.ds(m * 512, 512)]
                    first = True
                    if bias_k:
                        nc.tensor.matmul(reg, bk_sb[0:1, bass.ds(m * 128, 128)],
                                         ones_sb[0:1, :], start=True, stop=False)
                        first = False
                    for dc in range(8):
                        nc.tensor.matmul(
                            reg,
                            wk_sb[:, dc, bass.ds(m * 128, 128)],
                            st[:, dc, :],
                            start=first, stop=(dc == 7))
                        first = False
                ps_copy(dst[:, :, :], ps[:].rearrange("p (m s) -> p m s", m=2))

            # V projection (natural layout)
            psv = ps_pool.tile([128, 1024], F32, tag="ps", name=f"psv{c}")
            for blk in range(4):
                reg = psv[:, bass.ds(blk * 256, 256)]
                first = True
                if bias_v:
                    nc.tensor.matmul(reg, ones_sb[0:1, 0:128], bv_sb[0:1, :],
                                     start=True, stop=False)
                    first = False
                for dc in range(8):
                    nc.tensor.matmul(
                        reg,
                        vst[:, dc, bass.ds(blk * 128, 128)],
                        wv_sb[:, dc, :],
                        start=first, stop=(dc == 7))
                    first = False
            ps_copy(v_tiles[c][:, :, :, 0:64],
                    psv[:].rearrange("p (b h e) -> p b h e", b=4, h=HPC))

        # ---- phase 2: attention per head -----------------------------
        full_grid = mode != "causal"

        for hl in range(HPC):
            m = hl // 2
            p0 = 64 * (hl % 2)
            ut = ut_pool.tile([128, S], F32, tag="ut", name=f"ut{hl}")

            if full_grid:
                steps = [(j, w) for j in range(16) for w in range(2)]
            else:
                steps = [(j, w) for j in range(16) for w in range(j // 8, 2)]

            win_ps = {}
            win_exp = {}
            bc_tiles = {}

            def emit_scores(t):
                j, w = t
                ps = ps_pool.tile([128, 1024], F32, tag="ps", name=f"sc{hl}_{j}_{w}")
                win_ps[t] = ps
                a0 = 1024 * w if full_grid else max(128 * j, 1024 * w)
                if mode == "general":
                    mt = mask_pool.tile([128, 1024], BF16, tag="mask",
                                        name=f"mt{hl}_{j}_{w}")
                    nc.sync.dma_start(
                        mt[:, a0 - 1024 * w:],
                        maskT_d[bass.ds(128 * j, 128), bass.ds(a0, 1024 * (w + 1) - a0)])
                for sl_i in range(2):
                    lo = 1024 * w + 512 * sl_i
                    hi = lo + 512
                    if hi <= a0:
                        continue
                    nlo = max(lo, a0)
                    reg = ps[:, bass.ds(nlo - 1024 * w, hi - nlo)]
                    cq, off = nlo // 512, nlo % 512
                    rhs = qt_tiles[cq][p0:p0 + 64, m, bass.ds(off, hi - nlo)]
                    lhsT = kt_tiles[j // 4][p0:p0 + 64, m, bass.ds(128 * (j % 4), 128)]
                    diag_here = (mode == "causal") and lo <= 128 * j < hi
                    mask_here = (mode == "general")
                    nc.tensor.matmul(reg, lhsT, rhs, start=True,
                                     stop=not (diag_here or mask_here))
                    if diag_here:
                        doff = 128 * j - 1024 * w
                        nc.tensor.matmul(ps[:, bass.ds(doff, 128)], ident[:], dmask[:],
                                         start=False, stop=True)
                    elif mask_here:
                        nc.tensor.matmul(reg, ident[:],
                                         mt[:, bass.ds(nlo - 1024 * w, hi - nlo)],
                                         start=False, stop=True)

            def emit_exp(t):
                j, w = t
                ps = win_ps[t]
                a0 = 1024 * w if full_grid else max(128 * j, 1024 * w)
                off = a0 - 1024 * w
                et = exp_pool.tile([128, 1024], BF16, tag="exp", name=f"e{hl}_{j}_{w}")
                win_exp[t] = et
                nc.scalar.activation(et[:, off:1024], ps[:, off:1024],
                                     mybir.ActivationFunctionType.Exp, scale=S_INV)

            def emit_pv(t):
                j, w = t
                et = win_exp.pop(t)
                win_ps.pop(t)
                a0 = 1024 * w if full_grid else max(128 * j, 1024 * w)
                for sl_i in range(2):
                    lo = 1024 * w + 512 * sl_i
                    hi = lo + 512
                    if hi <= a0:
                        continue
                    nlo = max(lo, a0)
                    r = nlo // 512  # absolute 512-col region of UT
                    if full_grid:
                        start, stop = (j == 0), (j == 15)
                    else:
                        start, stop = (j == 0), (j == 4 * r + 3)
                    nc.tensor.matmul(
                        ut[0:65, bass.ds(nlo, hi - nlo)],
                        v_tiles[j // 4][:, j % 4, hl, 0:65],
                        et[:, bass.ds(nlo - 1024 * w, hi - nlo)],
                        start=start, stop=stop)

            def emit_norm_chain(r):
                """sums[512r:512r+512] -> reciprocal -> broadcast [64, 512]."""
                sm = sums_pool.tile([1, 512], F32, tag="sums", name=f"sm{hl}_{r}")
                nc.scalar.copy(sm[:], ut[64:65, bass.ds(512 * r, 512)])
                srt = srt_pool.tile([128, 4], F32, tag="srt", name=f"srt{hl}_{r}")
                nc.sync.dma_start(srt[:], sm[0:1, :])
                nc.vector.reciprocal(srt[:], srt[:])
                rcb = rcb_pool.tile([1, 512], F32, tag="rcb", name=f"rcb{hl}_{r}")
                nc.sync.dma_start(rcb[0:1, :], srt[:])
                bc = bc_pool.tile([64, 512], F32, tag="bc", name=f"bc{hl}_{r}")
                nc.gpsimd.partition_broadcast(bc[:], rcb[:], channels=64)
                bc_tiles[r] = bc

            # last step index touching region r
            if full_grid:
                region_done_at = {r: (15, r // 2) for r in range(4)}
            else:
                region_done_at = {r: (4 * r + 3, r // 2) for r in range(4)}

            emit_scores(steps[0])
            for i, t in enumerate(steps):
                if i + 1 < len(steps):
                    emit_scores(steps[i + 1])
                emit_exp(t)
                emit_pv(t)
                for r in range(4):
                    if region_done_at[r] == t:
                        emit_norm_chain(r)

            # normalize: oT[hd, sq] = UT[0:64] * (1 / UT[64]), per region
            if p0 == 0:
                dst = oT_sb[0:64, m, :]
            else:
                ott = ottmp_pool.tile([64, S], BF16, tag="ottmp", name=f"ott{hl}")
                dst = ott[:, :]
            for r in range(4):
                nc.vector.tensor_mul(
                    dst[:, bass.ds(512 * r, 512)],
                    ut[0:64, bass.ds(512 * r, 512)],
                    bc_tiles[r][:, :])
            if p0:
                nc.sync.dma_start(oT_sb[64:128, m, :], ott[:, :])

        # ---- phase 3: output projection ------------------------------
        for sb in range(16):
            ps = ps_pool.tile([128, 1024], F32, tag="ps", name=f"pso{sb}")
            for nh in range(2):
                reg = ps[:, bass.ds(nh * 512, 512)]
                for m in range(2):
                    nc.tensor.matmul(
                        reg,
                        oT_sb[:, m, bass.ds(sb * 128, 128)],
                        wo_sb[:, m, bass.ds(nh * 512, 512)],
                        start=(m == 0), stop=(m == 1))
            ob = outsb_pool.tile([128, D], F32, tag="outsb", name=f"ob{sb}")
            ps_copy(ob[:], ps[:])
            nc.sync.dma_start(out_d[bass.ds(sb * 128, 128), :], ob[:])

    nc.compile()
    return nc


_VONES = None
_ONES1 = np.ones((1, 512), dtype=np.float32)

_CACHE = {}


def _precision():
    return os.environ.get("MHA_PRECISION", "bf16")


def _get_nc(mode, bias_k, bias_v):
    key = (mode, bias_k, bias_v, _precision())
    if key not in _CACHE:
        _CACHE[key] = _build(mode, bias_k, bias_v, _precision())
    return _CACHE[key]


def make_in_maps(q, k, v, mask, Wk, bk, Wv, bv, Wo, bo):
    """Host-side sharding. Returns (mode, bias flags, in_maps)."""
    import ml_dtypes

    global _VONES
    if _VONES is None:
        _VONES = np.ones((128, 16), dtype=ml_dtypes.bfloat16)
    ones1 = (_ONES1 if _precision() != "bf16"
             else _ONES1.astype(ml_dtypes.bfloat16))

    q = np.asarray(q, dtype=np.float32)
    k = np.asarray(k, dtype=np.float32)
    v = np.asarray(v, dtype=np.float32)
    Wk = np.asarray(Wk, dtype=np.float32)
    Wv = np.asarray(Wv, dtype=np.float32)
    Wo = np.asarray(Wo, dtype=np.float32)
    bk = np.asarray(bk, dtype=np.float32).reshape(-1)
    bv = np.asarray(bv, dtype=np.float32).reshape(-1)
    bo = np.asarray(bo, dtype=np.float32).reshape(-1)
    mask2d = np.asarray(mask, dtype=np.float32).reshape(S, S)

    if not mask2d.any():
        mode = "none"
    elif np.array_equal(mask2d, np.triu(np.ones((S, S), np.float32), 1)):
        mode = "causal"
    else:
        mode = "general"
    bias_k, bias_v, bias_o = bool(bk.any()), bool(bv.any()), bool(bo.any())

    qT = [np.ascontiguousarray(q[b].T) for b in range(B)]
    kT = [np.ascontiguousarray(k[b].T) for b in range(B)]
    vT = [np.ascontiguousarray(v[b].T) for b in range(B)]
    if mode == "general":
        # pre-scale so adding before the fused exp scale matches the
        # reference's post-scale add:  (raw + m)*S_INV == raw*S_INV + mask*(-1e9)
        maskT = np.ascontiguousarray(
            (mask2d.T * np.float32(-1e9 / S_INV)).astype(ml_dtypes.bfloat16))

    in_maps = []
    for core in range(NCORES):
        b, g = divmod(core, HPC)
        cs = slice(CW * g, CW * (g + 1))
        im = {
            "qT": qT[b], "kT": kT[b], "vT": vT[b],
            "wk": np.ascontiguousarray(Wk[:, cs]),
            "wv": np.ascontiguousarray(Wv[:, cs]),
            "wo": np.ascontiguousarray(Wo[cs, :]),
        }
        im["vones"] = _VONES
        if bias_k or bias_v:
            im["ones1"] = ones1
        if bias_k:
            im["bk"] = np.ascontiguousarray(bk[cs]).reshape(1, CW)
        if bias_v:
            im["bv"] = np.ascontiguousarray(bv[cs]).reshape(1, CW)
        if mode == "general":
            im["maskT"] = maskT
        in_maps.append(im)
    return mode, (bias_k, bias_v, bias_o), in_maps


def assemble(results, bo=None):
    """Sum per-core partial outputs into the full [B, S, D] output."""
    full = np.zeros((B, S, D), dtype=np.float32)
    for b in range(B):
        acc = results[4 * b]["out"].astype(np.float32)
        for c in range(4 * b + 1, 4 * b + 4):
            acc = acc + results[c]["out"]
        if bo is not None:
            acc = acc + bo
        full[b] = acc
    return full


def kernel(q, k, v, mask, Wk, bk, Wv, bv, Wo, bo):
    mode, (bias_k, bias_v, bias_o), in_maps = make_in_maps(
        q, k, v, mask, Wk, bk, Wv, bv, Wo, bo)
    nc = _get_nc(mode, bias_k, bias_v)
    res = bass_utils.run_bass_kernel_spmd(nc, in_maps, core_ids=list(range(NCORES)))
    bo_arr = np.asarray(bo, dtype=np.float32).reshape(-1) if bias_o else None
    return assemble(res.results, bo_arr)


# revision 13
# speedup vs baseline: 1.3467x; 1.0502x over previous
"""Multi-head attention (B=2, S=2048, D=1024, H=16) on 8 Trainium2 cores.

Sharding: data-parallel over the 2 batches x tensor-parallel over 4 groups
of 4 heads.  Core c handles batch c//4 and heads [4*(c%4) : 4*(c%4)+4]
(columns [256*(c%4) : +256] of Wk/Wv, same rows of Wo).  Each core produces
a partial [S, D] output (its heads' contribution to o @ Wo); the host sums
the 4 partials per batch (and adds bo once).

Per-core dataflow (bf16 matmul operands, fp32 PSUM accumulation):
  qT,kT,vT [D,S] fp32 (host-pre-transposed) are DMA-cast to bf16 on load.
  Projections produce QT,KT [128,2,S] (head-major rows) and V [sk,hd] with
  an extra ones column.  Attention per head in "scores-transposed" layout
  [sk_part, sq_free]: scoresT = KT_j^T @ QT; the causal diagonal adds a
  bf16 -480 lower-triangular tile into PSUM via an identity matmul; exp on
  ScalarE (scale folded in; no max subtraction - scores are O(6));
  UT[65, S] += Vaug_j^T @ expT accumulated in PSUM, row 64 = softmax
  denominators (from the ones column).  Normalization is region-wise
  (512 cols at a time, as soon as that region's last k-block lands):
  sums -> DMA reshape [1,512]->[128,4] -> cheap DVE reciprocal -> DMA back
  -> gpsimd partition_broadcast -> one DVE multiply into oT [d_part, sq].
  Final: out = oT^T @ Wo per 128-row block, fp32 DMA to HBM.
"""

import itertools
import os
from contextlib import ExitStack

import numpy as np

import concourse.bass as bass
import concourse.tile as tile
from concourse import bacc, bass_utils, mybir
from concourse.masks import make_identity

B, S, D, H = 2, 2048, 1024, 16
HD = D // H            # 64
NCORES = 8
HPC = 4                # heads per core
CW = HPC * HD          # 256 weight cols per core
NCH = 4                # sequence chunks of 512
MASKVAL = -480.0       # additive pre-scale causal mask value (exp -> ~e-60)
S_INV = float(1.0 / (np.sqrt(np.float32(HD)) + np.float32(1e-8)))

F32 = mybir.dt.float32
F32R = mybir.dt.float32r
BF16 = mybir.dt.bfloat16


def _build(mode: str, bias_k: bool, bias_v: bool, precision: str = "bf16"):
    """Build + compile the SPMD program.

    mode: 'causal' | 'none' | 'general'
    precision: 'bf16' (everything bf16) or 'mixed' (fp32r projections).
    """
    nc = bacc.Bacc("TRN2", target_bir_lowering=False, debug=False,
                   num_devices=NCORES)
    xdt = BF16 if precision == "bf16" else F32R
    in_dt = F32 if precision == "bf16" else F32R  # dram decl for x/w inputs

    qT_d = nc.dram_tensor("qT", [D, S], in_dt, kind="ExternalInput").ap()
    kT_d = nc.dram_tensor("kT", [D, S], in_dt, kind="ExternalInput").ap()
    vT_d = nc.dram_tensor("vT", [D, S], in_dt, kind="ExternalInput").ap()
    wk_d = nc.dram_tensor("wk", [D, CW], in_dt, kind="ExternalInput").ap()
    wv_d = nc.dram_tensor("wv", [D, CW], in_dt, kind="ExternalInput").ap()
    wo_d = nc.dram_tensor("wo", [CW, D], F32, kind="ExternalInput").ap()
    bk_d = nc.dram_tensor("bk", [1, CW], in_dt, kind="ExternalInput").ap() if bias_k else None
    bv_d = nc.dram_tensor("bv", [1, CW], in_dt, kind="ExternalInput").ap() if bias_v else None
    maskT_d = (nc.dram_tensor("maskT", [S, S], BF16, kind="ExternalInput").ap()
               if mode == "general" else None)
    vones_d = nc.dram_tensor("vones", [128, 16], BF16, kind="ExternalInput").ap()
    ones1_d = (nc.dram_tensor("ones1", [1, 512], xdt, kind="ExternalInput").ap()
               if (bias_k or bias_v) else None)
    out_d = nc.dram_tensor("out", [S, D], F32, kind="ExternalOutput").ap()

    def load(dst, src):
        """DMA load, casting via SWDGE when dtypes differ."""
        if dst.dtype != src.dtype:
            nc.gpsimd.dma_start(dst, src)
        else:
            nc.sync.dma_start(dst, src)

    with tile.TileContext(nc) as tc, ExitStack() as ctx:
        sb1 = ctx.enter_context(tc.tile_pool(name="persist", bufs=1))
        qt_pool = ctx.enter_context(tc.tile_pool(name="qt", bufs=NCH))
        kt_pool = ctx.enter_context(tc.tile_pool(name="kt", bufs=NCH))
        v_pool = ctx.enter_context(tc.tile_pool(name="v", bufs=NCH))
        stage_pool = ctx.enter_context(tc.tile_pool(name="stage", bufs=4))
        exp_pool = ctx.enter_context(tc.tile_pool(name="exp", bufs=4))
        sums_pool = ctx.enter_context(tc.tile_pool(name="sums", bufs=4))
        srt_pool = ctx.enter_context(tc.tile_pool(name="srt", bufs=4))
        rcb_pool = ctx.enter_context(tc.tile_pool(name="rcb", bufs=4))
        bc_pool = ctx.enter_context(tc.tile_pool(name="bc", bufs=5))
        ottmp_pool = ctx.enter_context(tc.tile_pool(name="ottmp", bufs=1))
        outsb_pool = ctx.enter_context(tc.tile_pool(name="outsb", bufs=2))
        ps_pool = ctx.enter_context(tc.tile_pool(name="ps", bufs=4, space="PSUM"))
        ut_pool = ctx.enter_context(tc.tile_pool(name="ut", bufs=1, space="PSUM"))
        if mode == "general":
            mask_pool = ctx.enter_context(tc.tile_pool(name="mask", bufs=3))

        # ---- constants / weights -------------------------------------
        wk_sb = sb1.tile([128, 8, CW], xdt)
        load(wk_sb[:], wk_d.rearrange("(c p) n -> p c n", p=128))
        wv_sb = sb1.tile([128, 8, CW], xdt)
        load(wv_sb[:], wv_d.rearrange("(c p) n -> p c n", p=128))
        wo_sb = sb1.tile([128, 2, D], BF16)
        load(wo_sb[:], wo_d.rearrange("(m p) n -> p m n", p=128))
        if bias_k:
            bk_sb = sb1.tile([1, CW], xdt)
            load(bk_sb[:], bk_d[:])
        if bias_v:
            bv_sb = sb1.tile([1, CW], xdt)
            load(bv_sb[:], bv_d[:])
        if bias_k or bias_v:
            ones_sb = sb1.tile([1, 512], xdt)
            nc.sync.dma_start(ones_sb[:], ones1_d[:])
        if mode != "none":
            ident = sb1.tile([128, 128], BF16)
            make_identity(nc, ident[:])
        if mode == "causal":
            # dmask[p, f] = MASKVAL where f < p (sq < sk), else 0
            dmask = sb1.tile([128, 128], BF16)
            nc.gpsimd.memset(dmask[:], 0.0)
            nc.gpsimd.affine_select(
                out=dmask[:], in_=dmask[:],
                compare_op=mybir.AluOpType.is_ge,
                fill=MASKVAL, base=0,
                pattern=[[1, 128]], channel_multiplier=-1,
            )

        # V tiles: [128 sk, 4 blk, 4 head, 66] - col 64 is the ones column
        v_tiles = [v_pool.tile([128, 4, HPC, 66], BF16, tag="v", name=f"v{c}")
                   for c in range(NCH)]
        for c in range(NCH):
            nc.sync.dma_start(v_tiles[c][:, :, :, 64:65],
                              vones_d[:].rearrange("p (b h e) -> p b h e", b=4, h=HPC))
        qt_tiles = [qt_pool.tile([128, 2, 512], BF16, tag="qt", name=f"qt{c}")
                    for c in range(NCH)]
        kt_tiles = [kt_pool.tile([128, 2, 512], BF16, tag="kt", name=f"kt{c}")
                    for c in range(NCH)]
        oT_sb = sb1.tile([128, 2, S], BF16)

        copy_engines = itertools.cycle([nc.scalar, nc.vector])

        def ps_copy(dst, src):
            eng = next(copy_engines)
            if eng is nc.scalar:
                nc.scalar.copy(dst, src)
            else:
                nc.vector.tensor_copy(dst, src)

        # ---- phase 1: projections ------------------------------------
        for c in range(NCH):
            sl = bass.ds(c * 512, 512)
            kst = stage_pool.tile([128, 8, 512], xdt, tag="stage", name=f"kst{c}")
            load(kst[:], kT_d.rearrange("(cc p) s -> p cc s", p=128)[:, :, sl])
            vst = stage_pool.tile([128, 8, 512], xdt, tag="stage", name=f"vst{c}")
            load(vst[:], vT_d.rearrange("(cc p) s -> p cc s", p=128)[:, :, sl])
            qst = stage_pool.tile([128, 8, 512], xdt, tag="stage", name=f"qst{c}")
            load(qst[:], qT_d.rearrange("(cc p) s -> p cc s", p=128)[:, :, sl])

            # KT / QT projections (transposed layout, 2 m-halves of 128)
            for ti, (st, dst) in enumerate(((kst, kt_tiles[c]), (qst, qt_tiles[c]))):
                for m in range(2):
                    ps = ps_pool.tile([128, 512], F32, tag="ps", name=f"psp{c}_{ti}_{m}")
                    first = True
                    if bias_k:
                        nc.tensor.matmul(ps[:], bk_sb[0:1, bass.ds(m * 128, 128)],
                                         ones_sb[0:1, :], start=True, stop=False)
                        first = False
                    for dc in range(8):
                        nc.tensor.matmul(
                            ps[:],
                            wk_sb[:, dc, bass.ds(m * 128, 128)],
                            st[:, dc, :],
                            start=first, stop=(dc == 7))
                        first = False
                    ps_copy(dst[:, m, :], ps[:])

            # V projection (natural layout)
            for half in range(2):
                psv = ps_pool.tile([128, 512], F32, tag="ps", name=f"psv{c}_{half}")
                for loc in range(2):
                    blk = 2 * half + loc
                    reg = psv[:, bass.ds(loc * 256, 256)]
                    first = True
                    if bias_v:
                        nc.tensor.matmul(reg, ones_sb[0:1, 0:128], bv_sb[0:1, :],
                                         start=True, stop=False)
                        first = False
                    for dc in range(8):
                        nc.tensor.matmul(
                            reg,
                            vst[:, dc, bass.ds(blk * 128, 128)],
                            wv_sb[:, dc, :],
                            start=first, stop=(dc == 7))
                        first = False
                ps_copy(v_tiles[c][:, bass.ds(2 * half, 2), :, 0:64],
                        psv[:].rearrange("p (b h e) -> p b h e", b=2, h=HPC))

        # ---- phase 2: attention per head -----------------------------
        full_grid = mode != "causal"

        for hl in range(HPC):
            m = hl // 2
            p0 = 64 * (hl % 2)
            ut = ut_pool.tile([128, S], F32, tag="ut", name=f"ut{hl}")

            if full_grid:
                steps = [(j, r) for j in range(16) for r in range(4)]
            else:
                steps = [(j, r) for j in range(16) for r in range(j // 4, 4)]

            win_ps = {}
            win_exp = {}
            bc_tiles = {}

            def emit_scores(t):
                j, r = t
                ps = ps_pool.tile([128, 512], F32, tag="ps", name=f"sc{hl}_{j}_{r}")
                win_ps[t] = ps
                lo, hi = 512 * r, 512 * r + 512
                nlo = lo if full_grid else max(128 * j, lo)
                n = hi - nlo
                if mode == "general":
                    mt = mask_pool.tile([128, 512], BF16, tag="mask",
                                        name=f"mt{hl}_{j}_{r}")
                    nc.sync.dma_start(
                        mt[:, nlo - lo:],
                        maskT_d[bass.ds(128 * j, 128), bass.ds(nlo, n)])
                reg = ps[:, bass.ds(nlo - lo, n)]
                rhs = qt_tiles[r][p0:p0 + 64, m, bass.ds(nlo % 512, n)]
                lhsT = kt_tiles[j // 4][p0:p0 + 64, m, bass.ds(128 * (j % 4), 128)]
                diag_here = (mode == "causal") and lo <= 128 * j < hi
                mask_here = (mode == "general")
                nc.tensor.matmul(reg, lhsT, rhs, start=True,
                                 stop=not (diag_here or mask_here))
                if diag_here:
                    nc.tensor.matmul(ps[:, bass.ds(128 * j - lo, 128)],
                                     ident[:], dmask[:], start=False, stop=True)
                elif mask_here:
                    nc.tensor.matmul(reg, ident[:], mt[:, bass.ds(nlo - lo, n)],
                                     start=False, stop=True)

            def emit_exp(t):
                j, r = t
                ps = win_ps[t]
                lo = 512 * r
                off = 0 if full_grid else max(128 * j - lo, 0)
                et = exp_pool.tile([128, 512], BF16, tag="exp", name=f"e{hl}_{j}_{r}")
                win_exp[t] = et
                nc.scalar.activation(et[:, off:512], ps[:, off:512],
                                     mybir.ActivationFunctionType.Exp, scale=S_INV)

            def emit_pv(t):
                j, r = t
                et = win_exp.pop(t)
                win_ps.pop(t)
                lo = 512 * r
                off = 0 if full_grid else max(128 * j - lo, 0)
                if full_grid:
                    start, stop = (j == 0), (j == 15)
                else:
                    start, stop = (j == 0), (j == 4 * r + 3)
                nc.tensor.matmul(
                    ut[0:65, bass.ds(lo + off, 512 - off)],
                    v_tiles[j // 4][:, j % 4, hl, 0:65],
                    et[:, bass.ds(off, 512 - off)],
                    start=start, stop=stop)

            def emit_norm_chain(r):
                """sums[512r:512r+512] -> reciprocal -> broadcast [64, 512]."""
                sm = sums_pool.tile([1, 512], F32, tag="sums", name=f"sm{hl}_{r}")
                nc.scalar.copy(sm[:], ut[64:65, bass.ds(512 * r, 512)])
                srt = srt_pool.tile([128, 4], F32, tag="srt", name=f"srt{hl}_{r}")
                nc.sync.dma_start(srt[:], sm[0:1, :])
                nc.vector.reciprocal(srt[:], srt[:])
                rcb = rcb_pool.tile([1, 512], F32, tag="rcb", name=f"rcb{hl}_{r}")
                nc.sync.dma_start(rcb[0:1, :], srt[:])
                bc = bc_pool.tile([64, 512], F32, tag="bc", name=f"bc{hl}_{r}")
                nc.gpsimd.partition_broadcast(bc[:], rcb[:], channels=64)
                bc_tiles[r] = bc

            # last step index touching region r
            if full_grid:
                region_done_at = {r: (15, r) for r in range(4)}
            else:
                region_done_at = {r: (4 * r + 3, r) for r in range(4)}

            LOOKAHEAD = 2
            for i in range(min(LOOKAHEAD, len(steps))):
                emit_scores(steps[i])
            for i, t in enumerate(steps):
                if i + LOOKAHEAD < len(steps):
                    emit_scores(steps[i + LOOKAHEAD])
                emit_exp(t)
                emit_pv(t)
                for r in range(4):
                    if region_done_at[r] == t:
                        emit_norm_chain(r)

            # normalize: oT[hd, sq] = UT[0:64] * (1 / UT[64]), per region
            if p0 == 0:
                dst = oT_sb[0:64, m, :]
            else:
                ott = ottmp_pool.tile([64, S], BF16, tag="ottmp", name=f"ott{hl}")
                dst = ott[:, :]
            for r in range(4):
                nc.vector.tensor_mul(
                    dst[:, bass.ds(512 * r, 512)],
                    ut[0:64, bass.ds(512 * r, 512)],
                    bc_tiles[r][:, :])
            if p0:
                nc.sync.dma_start(oT_sb[64:128, m, :], ott[:, :])

        # ---- phase 3: output projection ------------------------------
        for sb in range(16):
            ob = outsb_pool.tile([128, D], F32, tag="outsb", name=f"ob{sb}")
            for nh in range(2):
                ps = ps_pool.tile([128, 512], F32, tag="ps", name=f"pso{sb}_{nh}")
                for m in range(2):
                    nc.tensor.matmul(
                        ps[:],
                        oT_sb[:, m, bass.ds(sb * 128, 128)],
                        wo_sb[:, m, bass.ds(nh * 512, 512)],
                        start=(m == 0), stop=(m == 1))
                ps_copy(ob[:, bass.ds(nh * 512, 512)], ps[:])
            nc.sync.dma_start(out_d[bass.ds(sb * 128, 128), :], ob[:])

    nc.compile()
    return nc


_VONES = None
_ONES1 = np.ones((1, 512), dtype=np.float32)

_CACHE = {}


def _precision():
    return os.environ.get("MHA_PRECISION", "bf16")


def _get_nc(mode, bias_k, bias_v):
    key = (mode, bias_k, bias_v, _precision())
    if key not in _CACHE:
        _CACHE[key] = _build(mode, bias_k, bias_v, _precision())
    return _CACHE[key]


def make_in_maps(q, k, v, mask, Wk, bk, Wv, bv, Wo, bo):
    """Host-side sharding. Returns (mode, bias flags, in_maps)."""
    import ml_dtypes

    global _VONES
    if _VONES is None:
        _VONES = np.ones((128, 16), dtype=ml_dtypes.bfloat16)
    ones1 = (_ONES1 if _precision() != "bf16"
             else _ONES1.astype(ml_dtypes.bfloat16))

    q = np.asarray(q, dtype=np.float32)
    k = np.asarray(k, dtype=np.float32)
    v = np.asarray(v, dtype=np.float32)
    Wk = np.asarray(Wk, dtype=np.float32)
    Wv = np.asarray(Wv, dtype=np.float32)
    Wo = np.asarray(Wo, dtype=np.float32)
    bk = np.asarray(bk, dtype=np.float32).reshape(-1)
    bv = np.asarray(bv, dtype=np.float32).reshape(-1)
    bo = np.asarray(bo, dtype=np.float32).reshape(-1)
    mask2d = np.asarray(mask, dtype=np.float32).reshape(S, S)

    if not mask2d.any():
        mode = "none"
    elif np.array_equal(mask2d, np.triu(np.ones((S, S), np.float32), 1)):
        mode = "causal"
    else:
        mode = "general"
    bias_k, bias_v, bias_o = bool(bk.any()), bool(bv.any()), bool(bo.any())

    qT = [np.ascontiguousarray(q[b].T) for b in range(B)]
    kT = [np.ascontiguousarray(k[b].T) for b in range(B)]
    vT = [np.ascontiguousarray(v[b].T) for b in range(B)]
    if mode == "general":
        # pre-scale so adding before the fused exp scale matches the
        # reference's post-scale add:  (raw + m)*S_INV == raw*S_INV + mask*(-1e9)
        maskT = np.ascontiguousarray(
            (mask2d.T * np.float32(-1e9 / S_INV)).astype(ml_dtypes.bfloat16))

    in_maps = []
    for core in range(NCORES):
        b, g = divmod(core, HPC)
        cs = slice(CW * g, CW * (g + 1))
        im = {
            "qT": qT[b], "kT": kT[b], "vT": vT[b],
            "wk": np.ascontiguousarray(Wk[:, cs]),
            "wv": np.ascontiguousarray(Wv[:, cs]),
            "wo": np.ascontiguousarray(Wo[cs, :]),
        }
        im["vones"] = _VONES
        if bias_k or bias_v:
            im["ones1"] = ones1
        if bias_k:
            im["bk"] = np.ascontiguousarray(bk[cs]).reshape(1, CW)
        if bias_v:
            im["bv"] = np.ascontiguousarray(bv[cs]).reshape(1, CW)
        if mode == "general":
            im["maskT"] = maskT
        in_maps.append(im)
    return mode, (bias_k, bias_v, bias_o), in_maps


def assemble(results, bo=None):
    """Sum per-core partial outputs into the full [B, S, D] output."""
    full = np.zeros((B, S, D), dtype=np.float32)
    for b in range(B):
        acc = results[4 * b]["out"].astype(np.float32)
        for c in range(4 * b + 1, 4 * b + 4):
            acc = acc + results[c]["out"]
        if bo is not None:
            acc = acc + bo
        full[b] = acc
    return full


def kernel(q, k, v, mask, Wk, bk, Wv, bv, Wo, bo):
    mode, (bias_k, bias_v, bias_o), in_maps = make_in_maps(
        q, k, v, mask, Wk, bk, Wv, bv, Wo, bo)
    nc = _get_nc(mode, bias_k, bias_v)
    res = bass_utils.run_bass_kernel_spmd(nc, in_maps, core_ids=list(range(NCORES)))
    bo_arr = np.asarray(bo, dtype=np.float32).reshape(-1) if bias_o else None
    return assemble(res.results, bo_arr)
